# revision 1
# baseline (speedup 1.0000x reference)
# Bidirectional LSTM (B=512, T=256, E=256, U=512) + MLP + softmax(V=10000)
# on 8 trn2 NeuronCores.
#
# Distribution: data-parallel over batch x direction. Cores 0-3 run the
# forward LSTM on batch slices of 128; cores 4-7 run the backward LSTM on the
# same slices (time-reversed token stream, supplied via the gather index
# table, so the SPMD program is identical on every core). The final MLP needs
# h_fw and h_bw of the same rows, so core pairs (i, i+4) AllReduce their
# partial h @ W1-half products and then redundantly compute the same 128
# output rows; the host keeps the fw copies.
#
# Per step t (one core, batch 128):
#   gates[128,2048] (PSUM, fp32) = x_t @ Wx + h_{t-1} @ Wh   as lhsT.T @ rhs
#     with the *data* transposed as stationary operand (xT from a transposing
#     embedding dma_gather; hT from a per-step PE transpose) and the bf16
#     weights streaming.
#   i,f,o = sigmoid(gates[:,0:1536]); g = tanh(gates[:,1536:2048])  (ScalarE,
#     gate columns pre-permuted to [i f o g] on the host)
#   c = f*c + i*g (DVE, fp32 state);  h = o * tanh(c)  (bf16)
import os
import numpy as np
import ml_dtypes

B, T, E, U, V = 512, 256, 256, 512, 10000
G4 = 4 * U
NCORES = 8
BC = 128              # batch rows per core
NK_X = E // 128       # 2 contraction tiles for x
NK_H = U // 128       # 4 contraction tiles for h
NBW = int(os.environ.get("KERNEL_NBW", "512"))  # matmul n-block width
NB = G4 // NBW        # 4 n-blocks
TOK = BC * T          # 32768 tokens gathered per core
T_STEPS = int(os.environ.get("KERNEL_T", T))
CHUNK_STEPS = 4   # 512 tokens per dma_gather (>512 idxs crashes SWDGE)
CHUNK_TOK = BC * CHUNK_STEPS
NCHUNK = (T_STEPS + CHUNK_STEPS - 1) // CHUNK_STEPS
VCH = 500             # logits chunk width
NVCH = V // VCH

_prog_cache = {}


def _build_program(with_gate_bias: bool, with_b2: bool):
    import concourse.bass as bass
    import concourse.mybir as mybir
    import concourse.tile as tile
    from concourse import bacc
    from concourse.masks import make_identity
    from contextlib import ExitStack

    f32 = mybir.dt.float32
    bf16 = mybir.dt.bfloat16
    i16 = mybir.dt.int16
    AF = mybir.ActivationFunctionType

    nc = bacc.Bacc("TRN2", debug=False, enable_asserts=False, num_devices=NCORES)

    emb_d = nc.dram_tensor("emb16", [V, E], bf16, kind="ExternalInput").ap()
    idx_d = nc.dram_tensor("idx16", [128, TOK // 16], i16, kind="ExternalInput").ap()
    wx_d = nc.dram_tensor("wx", [NK_X, 128, G4], bf16, kind="ExternalInput").ap()
    wh_d = nc.dram_tensor("wh", [NK_H, 128, G4], bf16, kind="ExternalInput").ap()
    w1_d = nc.dram_tensor("w1h", [NK_H, 128, 64], bf16, kind="ExternalInput").ap()
    w2_d = nc.dram_tensor("w2", [64, V], bf16, kind="ExternalInput").ap()
    b1_d = nc.dram_tensor("b1bc", [128, 64], f32, kind="ExternalInput").ap()
    if with_gate_bias:
        bg_d = nc.dram_tensor("bgbc", [128, G4], f32, kind="ExternalInput").ap()
    if with_b2:
        b2_d = nc.dram_tensor("b2bc", [128, V], f32, kind="ExternalInput").ap()
    out_d = nc.dram_tensor("out", [BC, V], f32, kind="ExternalOutput").ap()

    with tile.TileContext(nc) as tc, ExitStack() as ctx:
        const = ctx.enter_context(tc.tile_pool(name="const", bufs=1))
        gpool = ctx.enter_context(tc.tile_pool(name="gather", bufs=3))
        work = ctx.enter_context(tc.tile_pool(name="work", bufs=2))
        psum = ctx.enter_context(tc.tile_pool(name="psum", bufs=1, space="PSUM"))
        dram = ctx.enter_context(tc.tile_pool(name="dram", bufs=1, space="DRAM"))

        wx_sb = const.tile([128, NK_X, G4], bf16)
        for k in range(NK_X):
            nc.sync.dma_start(wx_sb[:, k, :], wx_d[k])
        wh_sb = const.tile([128, NK_H, G4], bf16)
        for k in range(NK_H):
            nc.sync.dma_start(wh_sb[:, k, :], wh_d[k])
        w1_sb = const.tile([128, NK_H, 64], bf16)
        for k in range(NK_H):
            nc.sync.dma_start(w1_sb[:, k, :], w1_d[k])
        w2_sb = const.tile([64, V], bf16)
        nc.sync.dma_start(w2_sb[:], w2_d[:])
        b1_sb = const.tile([128, 64], f32)
        nc.sync.dma_start(b1_sb[:], b1_d[:])
        # DVE pre-copy so downstream tensor_tensor ops have a same-engine dep
        # (walrus TT format has a single sync-wait slot).
        b1c = const.tile([128, 64], f32)
        nc.vector.tensor_copy(b1c[:], b1_sb[:])
        if with_gate_bias:
            bg_sb = const.tile([128, G4], f32)
            nc.sync.dma_start(bg_sb[:], bg_d[:])
            bgc = const.tile([128, G4], f32)
            nc.vector.tensor_copy(bgc[:], bg_sb[:])
        if with_b2:
            b2_sb = const.tile([128, V], f32)
            nc.sync.dma_start(b2_sb[:], b2_d[:])
        idx_sb = const.tile([128, TOK // 16], i16)
        nc.sync.dma_start(idx_sb[:], idx_d[:])
        ident = const.tile([128, 128], bf16)
        make_identity(nc, ident[:])
        c_sb = const.tile([128, U], f32)

        xg_tiles = {}

        def issue_gather(ci):
            xg = gpool.tile(
                [128, NK_X, CHUNK_TOK], bf16, tag="xg", name=f"xg{ci}"
            )
            nc.gpsimd.dma_gather(
                xg[:],
                emb_d[:],
                idx_sb[:, ci * (CHUNK_TOK // 16):(ci + 1) * (CHUNK_TOK // 16)],
                CHUNK_TOK,
                CHUNK_TOK,
                E,
                transpose=True,
            )
            xg_tiles[ci] = xg

        issue_gather(0)

        hT_prev = None
        for t in range(T_STEPS):
            ci = t // CHUNK_STEPS
            w = t % CHUNK_STEPS
            if w == 1 and ci + 1 < NCHUNK:
                issue_gather(ci + 1)
            xg = xg_tiles[ci]

            gates = psum.tile([128, G4], f32, tag="gates", name=f"gates{t}")
            n_kt = NK_X + (NK_H if hT_prev is not None else 0)
            ki = 0
            for k in range(NK_X):
                for n in range(NB):
                    nc.tensor.matmul(
                        gates[:, n * NBW:(n + 1) * NBW],
                        lhsT=xg[:, k, w * BC:(w + 1) * BC],
                        rhs=wx_sb[:, k, n * NBW:(n + 1) * NBW],
                        start=(ki == 0),
                        stop=(ki == n_kt - 1),
                    )
                ki += 1
            if hT_prev is not None:
                for k in range(NK_H):
                    for n in range(NB):
                        nc.tensor.matmul(
                            gates[:, n * NBW:(n + 1) * NBW],
                            lhsT=hT_prev[:, k * 128:(k + 1) * 128],
                            rhs=wh_sb[:, k, n * NBW:(n + 1) * NBW],
                            start=(ki == 0),
                            stop=(ki == n_kt - 1),
                        )
                    ki += 1
            if with_gate_bias:
                nc.vector.tensor_add(gates[:], gates[:], bgc[:])

            ifo = work.tile([128, 3 * U], bf16, tag="ifo", name=f"ifo{t}")
            nc.scalar.activation(ifo[:], gates[:, 0:3 * U], AF.Sigmoid)
            gg = work.tile([128, U], bf16, tag="gg", name=f"gg{t}")
            nc.scalar.activation(gg[:], gates[:, 3 * U:G4], AF.Tanh)

            if t == 0:
                # c = i*g (c starts at zero; avoids a memset feeding a TT)
                nc.vector.tensor_mul(c_sb[:], ifo[:, 0:U], gg[:])
            else:
                pp = work.tile([128, U], bf16, tag="pp", name=f"pp{t}")
                nc.vector.tensor_mul(pp[:], ifo[:, 0:U], gg[:])
                fc = work.tile([128, U], f32, tag="fc", name=f"fc{t}")
                nc.vector.tensor_mul(fc[:], ifo[:, U:2 * U], c_sb[:])
                nc.vector.tensor_add(c_sb[:], fc[:], pp[:])
            tct = work.tile([128, U], bf16, tag="tct", name=f"tct{t}")
            nc.scalar.activation(tct[:], c_sb[:], AF.Tanh)
            h = work.tile([128, U], bf16, tag="h", name=f"h{t}")
            nc.vector.tensor_mul(h[:], ifo[:, 2 * U:3 * U], tct[:])

            trp = psum.tile([128, U], bf16, tag="trp", bufs=2, name=f"trp{t}")
            for k in range(NK_H):
                nc.tensor.transpose(
                    trp[:, k * 128:(k + 1) * 128],
                    h[:, k * 128:(k + 1) * 128],
                    ident[:],
                )
            hT = work.tile([128, U], bf16, tag="hT", name=f"hT{t}")
            nc.vector.tensor_copy(hT[:], trp[:])
            hT_prev = hT

        if os.environ.get("KERNEL_STOP_AFTER", "") == "recur":
            nc.gpsimd.dma_start(out_d[:, 0:U], hT_prev[:])
        else:
            # ---- MLP head: P = h_final @ W1half -> pairwise AllReduce -> relu
            pps = psum.tile([128, 64], f32, tag="gates", name="pps")
            for k in range(NK_H):
                nc.tensor.matmul(
                    pps[:],
                    lhsT=hT_prev[:, k * 128:(k + 1) * 128],
                    rhs=w1_sb[:, k, :],
                    start=(k == 0),
                    stop=(k == NK_H - 1),
                )
            p_sb = work.tile([128, 64], f32, tag="p_sb", bufs=1)
            nc.vector.tensor_copy(p_sb[:], pps[:])
            cc_in = dram.tile([128, 64], f32, name="cc_in")
            cc_out = dram.tile([128, 64], f32, name="cc_out")
            nc.sync.dma_start(cc_in[:], p_sb[:])
            if os.environ.get("KERNEL_SKIP_CC"):
                nc.sync.dma_start(cc_out[:], cc_in[:])
            else:
                nc.gpsimd.collective_compute(
                    "AllReduce",
                    mybir.AluOpType.add,
                    replica_groups=[[0, 4], [1, 5], [2, 6], [3, 7]],
                    ins=[cc_in.opt()],
                    outs=[cc_out.opt()],
                )
            p2_sb = work.tile([128, 64], f32, tag="p2_sb", bufs=1)
            nc.sync.dma_start(p2_sb[:], cc_out[:])
            nc.vector.tensor_add(p2_sb[:], p2_sb[:], b1c[:])
            hid = work.tile([128, 64], bf16, tag="hid", bufs=1)
            nc.scalar.activation(hid[:], p2_sb[:], AF.Relu)

            hps = psum.tile([64, 128], bf16, tag="trp", bufs=2, name="hps")
            nc.tensor.transpose(hps[:], hid[:], ident[:])
            hidT = work.tile([64, 128], bf16, tag="hidT", bufs=1)
            nc.vector.tensor_copy(hidT[:], hps[:])

            logits = work.tile([128, V], f32, tag="logits", bufs=1)
            for vc in range(NVCH):
                lp = psum.tile([128, VCH], f32, tag="trp", bufs=2, name=f"lp{vc}")
                nc.tensor.matmul(
                    lp[:],
                    lhsT=hidT[:],
                    rhs=w2_sb[:, vc * VCH:(vc + 1) * VCH],
                    start=True,
                    stop=True,
                )
                nc.vector.tensor_copy(logits[:, vc * VCH:(vc + 1) * VCH], lp[:])
            if with_b2:
                nc.vector.tensor_add(logits[:], logits[:], b2_sb[:])

            negmax = work.tile([128, 1], f32, tag="negmax", bufs=1)
            nc.vector.reduce_max(
                negmax[:], logits[:], axis=mybir.AxisListType.X, negate=True
            )
            exps = work.tile([128, V], bf16, tag="exps", bufs=1)
            sume = work.tile([128, 1], f32, tag="sume", bufs=1)
            nc.scalar.activation(
                exps[:], logits[:], AF.Exp, bias=negmax[:], accum_out=sume[:]
            )
            rcp = work.tile([128, 1], f32, tag="rcp", bufs=1)
            nc.vector.reciprocal(rcp[:], sume[:])
            nc.vector.tensor_scalar_mul(logits[:], exps[:], rcp[:])
            nc.sync.dma_start(out_d[:], logits[:])

    nc.finalize()
    return nc


def _get_program(with_gate_bias: bool, with_b2: bool):
    key = (with_gate_bias, with_b2, T_STEPS)
    if key not in _prog_cache:
        _prog_cache[key] = _build_program(with_gate_bias, with_b2)
    return _prog_cache[key]


# gate column permutation: reference order [i f g o] -> kernel order [i f o g]
_PERM = np.concatenate(
    [np.arange(0, U), np.arange(U, 2 * U), np.arange(3 * U, 4 * U),
     np.arange(2 * U, 3 * U)]
)


def _pack_w(Wx, Wh, b):
    bf = ml_dtypes.bfloat16
    wxp = np.ascontiguousarray(
        Wx[:, _PERM].reshape(NK_X, 128, G4).astype(bf)
    )
    whp = np.ascontiguousarray(
        Wh[:, _PERM].reshape(NK_H, 128, G4).astype(bf)
    )
    bp = np.ascontiguousarray(b[_PERM].astype(np.float32))
    return wxp, whp, bp


def _make_idx(tokens_tmajor_flat):
    # dma_gather reads index i from [i % 16, i // 16]; the 16-partition index
    # block must be replicated for each of the 8 gpsimd cores (128 partitions).
    wrapped = tokens_tmajor_flat.astype(np.int16).reshape(-1, 16).T
    return np.ascontiguousarray(np.tile(wrapped, (8, 1)))


def prepare(inputs):
    """Build (nc, in_maps) for the 8 cores from full unsharded inputs."""
    bf = ml_dtypes.bfloat16
    sentence = np.asarray(inputs["sentence"])
    emb = np.asarray(inputs["emb"], np.float32)
    Wx_fw = np.asarray(inputs["Wx_fw"], np.float32)
    Wh_fw = np.asarray(inputs["Wh_fw"], np.float32)
    b_fw = np.asarray(inputs["b_fw"], np.float32)
    Wx_bw = np.asarray(inputs["Wx_bw"], np.float32)
    Wh_bw = np.asarray(inputs["Wh_bw"], np.float32)
    b_bw = np.asarray(inputs["b_bw"], np.float32)
    W1 = np.asarray(inputs["W1"], np.float32)
    b1 = np.asarray(inputs["b1"], np.float32)
    W2 = np.asarray(inputs["W2"], np.float32)
    b2 = np.asarray(inputs["b2"], np.float32)

    with_gate_bias = bool(np.any(b_fw) or np.any(b_bw))
    with_b2 = bool(np.any(b2))
    nc = _get_program(with_gate_bias, with_b2)

    emb16 = np.ascontiguousarray(emb.astype(bf))
    wx_f, wh_f, bg_f = _pack_w(Wx_fw, Wh_fw, b_fw)
    wx_b, wh_b, bg_b = _pack_w(Wx_bw, Wh_bw, b_bw)
    w1f = np.ascontiguousarray(W1[0:U].reshape(NK_H, 128, 64).astype(bf))
    w1b = np.ascontiguousarray(W1[U:2 * U].reshape(NK_H, 128, 64).astype(bf))
    w2p = np.ascontiguousarray(W2.astype(bf))
    b1bc = np.ascontiguousarray(np.broadcast_to(b1[None, :], (128, 64)).astype(np.float32))

    in_maps = []
    for c in range(NCORES):
        fw = c < 4
        rows = slice(128 * (c % 4), 128 * (c % 4) + 128)
        toks = sentence[rows][:, :T]
        if not fw:
            toks = toks[:, ::-1]
        flat = np.ascontiguousarray(toks.T).reshape(-1)  # t-major
        m = {
            "emb16": emb16,
            "idx16": _make_idx(flat),
            "wx": wx_f if fw else wx_b,
            "wh": wh_f if fw else wh_b,
            "w1h": w1f if fw else w1b,
            "w2": w2p,
            "b1bc": b1bc,
        }
        if with_gate_bias:
            bg = bg_f if fw else bg_b
            m["bgbc"] = np.ascontiguousarray(
                np.broadcast_to(bg[None, :], (128, G4)).astype(np.float32)
            )
        if with_b2:
            m["b2bc"] = np.ascontiguousarray(
                np.broadcast_to(b2[None, :], (128, V)).astype(np.float32)
            )
        in_maps.append(m)
    return nc, in_maps


def kernel(**inputs):
    from concourse.bass_utils import run_bass_kernel_spmd

    nc, in_maps = prepare(inputs)
    res = run_bass_kernel_spmd(
        nc, in_maps, core_ids=list(range(NCORES)),
        trace=bool(int(os.environ.get("KERNEL_TRACE", "0"))),
    )
    out = np.concatenate([res.results[c]["out"] for c in range(4)], axis=0)
    kernel.last_results = res
    return out.astype(np.float32)



# revision 2
# speedup vs baseline: 37.0057x; 37.0057x over previous
# Bidirectional LSTM (B=512, T=256, E=256, U=512) + MLP + softmax(V=10000)
# on 8 trn2 NeuronCores.
#
# Distribution: data-parallel over batch x direction. Cores 0-3 run the
# forward LSTM on batch slices of 128; cores 4-7 run the backward LSTM on the
# same slices (time-reversed token stream via the gather index table, so the
# SPMD program is identical on every core). Core pairs (i, i+4) AllReduce
# their partial h @ W1-half products; the host keeps the fw copies.
#
# v2 pipeline (per step, per core, batch 128):
#   gates live in 4 per-block PSUM tiles [128,512] f32 (blocks = i,g,f,o in
#   permuted column order), each tag double-buffered: the x-part matmuls for
#   step t+1 accumulate into the other slot DURING step t's activation tail,
#   so TensorE never idles and HAM stays un-throttled.
#   Per-block activations ([i] sigmoid -> [g] tanh -> [f] sigmoid -> [o]
#   sigmoid) start as soon as that block's h-matmuls finish, overlapping the
#   remaining h/x matmuls.  c = f*c + i*g (fp32, DVE); h = o*tanh(c) (bf16);
#   hT via 4 PE transposes into a borrowed PSUM slot + DVE copy.
import os
import numpy as np
import ml_dtypes

B, T, E, U, V = 512, 256, 256, 512, 10000
G4 = 4 * U
NCORES = 8
BC = 128              # batch rows per core
NK_X = E // 128       # 2 contraction tiles for x
NK_H = U // 128       # 4 contraction tiles for h
NBW = 512             # matmul n-block width (= one PSUM bank fp32)
NB = G4 // NBW        # 4 n-blocks: [i g f o] in permuted column order
TOK = BC * T          # 32768 tokens gathered per core
T_STEPS = int(os.environ.get("KERNEL_T", T))
CHUNK_STEPS = 4   # 512 tokens per dma_gather (>512 idxs crashes SWDGE)
CHUNK_TOK = BC * CHUNK_STEPS
NCHUNK = (T_STEPS + CHUNK_STEPS - 1) // CHUNK_STEPS
VCH = 500             # logits chunk width
NVCH = V // VCH

_prog_cache = {}


def _build_program(with_gate_bias: bool, with_b2: bool):
    import concourse.bass as bass
    import concourse.mybir as mybir
    import concourse.tile as tile
    from concourse import bacc
    from concourse.masks import make_identity
    from contextlib import ExitStack

    f32 = mybir.dt.float32
    bf16 = mybir.dt.bfloat16
    i16 = mybir.dt.int16
    AF = mybir.ActivationFunctionType

    nc = bacc.Bacc("TRN2", debug=False, enable_asserts=False, num_devices=NCORES)

    emb_d = nc.dram_tensor("emb16", [V, E], bf16, kind="ExternalInput").ap()
    idx_d = nc.dram_tensor("idx16", [128, TOK // 16], i16, kind="ExternalInput").ap()
    wx_d = nc.dram_tensor("wx", [NK_X, 128, G4], bf16, kind="ExternalInput").ap()
    wh_d = nc.dram_tensor("wh", [NK_H, 128, G4], bf16, kind="ExternalInput").ap()
    w1_d = nc.dram_tensor("w1h", [NK_H, 128, 64], bf16, kind="ExternalInput").ap()
    w2_d = nc.dram_tensor("w2", [64, V], bf16, kind="ExternalInput").ap()
    b1_d = nc.dram_tensor("b1bc", [128, 64], f32, kind="ExternalInput").ap()
    if with_gate_bias:
        bg_d = nc.dram_tensor("bgbc", [128, G4], f32, kind="ExternalInput").ap()
    if with_b2:
        b2_d = nc.dram_tensor("b2bc", [128, V], f32, kind="ExternalInput").ap()
    out_d = nc.dram_tensor("out", [BC, V], f32, kind="ExternalOutput").ap()

    with tile.TileContext(nc) as tc, ExitStack() as ctx:
        const = ctx.enter_context(tc.tile_pool(name="const", bufs=1))
        gpool = ctx.enter_context(tc.tile_pool(name="gather", bufs=3))
        work = ctx.enter_context(tc.tile_pool(name="work", bufs=2))
        psum = ctx.enter_context(tc.tile_pool(name="psum", bufs=2, space="PSUM"))
        dram = ctx.enter_context(tc.tile_pool(name="dram", bufs=1, space="DRAM"))

        wx_sb = const.tile([128, NK_X, G4], bf16)
        for k in range(NK_X):
            nc.sync.dma_start(wx_sb[:, k, :], wx_d[k])
        wh_sb = const.tile([128, NK_H, G4], bf16)
        for k in range(NK_H):
            nc.sync.dma_start(wh_sb[:, k, :], wh_d[k])
        w1_sb = const.tile([128, NK_H, 64], bf16)
        for k in range(NK_H):
            nc.sync.dma_start(w1_sb[:, k, :], w1_d[k])
        w2_sb = const.tile([64, V], bf16)
        nc.sync.dma_start(w2_sb[:], w2_d[:])
        b1_sb = const.tile([128, 64], f32)
        nc.sync.dma_start(b1_sb[:], b1_d[:])
        # DVE pre-copy so downstream tensor_tensor ops have a same-engine dep
        # (walrus TT format has a single sync-wait slot).
        b1c = const.tile([128, 64], f32)
        nc.vector.tensor_copy(b1c[:], b1_sb[:])
        if with_gate_bias:
            bg_sb = const.tile([128, G4], f32)
            nc.sync.dma_start(bg_sb[:], bg_d[:])
            bgc = const.tile([128, G4], f32)
            nc.vector.tensor_copy(bgc[:], bg_sb[:])
        if with_b2:
            b2_sb = const.tile([128, V], f32)
            nc.sync.dma_start(b2_sb[:], b2_d[:])
        idx_sb = const.tile([128, TOK // 16], i16)
        nc.sync.dma_start(idx_sb[:], idx_d[:])
        ident = const.tile([128, 128], bf16)
        make_identity(nc, ident[:])
        c_sb = const.tile([128, U], f32)

        xg_tiles = {}

        def issue_gather(ci):
            xg = gpool.tile(
                [128, NK_X, CHUNK_TOK], bf16, tag="xg", name=f"xg{ci}"
            )
            nc.gpsimd.dma_gather(
                xg[:],
                emb_d[:],
                idx_sb[:, ci * (CHUNK_TOK // 16):(ci + 1) * (CHUNK_TOK // 16)],
                CHUNK_TOK,
                CHUNK_TOK,
                E,
                transpose=True,
            )
            xg_tiles[ci] = xg

        issue_gather(0)

        GT = ["gi", "gg", "gf", "go"]

        def x_matmuls(t, first_stop):
            # x-part matmuls for step t into fresh per-block psum tiles.
            # first_stop: close the group (t==0 only; otherwise h matmuls
            # continue the accumulation at step t).
            ci = t // CHUNK_STEPS
            w = t % CHUNK_STEPS
            xg = xg_tiles[ci]
            tiles = []
            for nb in range(NB):
                gb = psum.tile([128, NBW], f32, tag=GT[nb], name=f"{GT[nb]}_{t}")
                for k in range(NK_X):
                    nc.tensor.matmul(
                        gb[:],
                        lhsT=xg[:, k, w * BC:(w + 1) * BC],
                        rhs=wx_sb[:, k, nb * NBW:(nb + 1) * NBW],
                        start=(k == 0),
                        stop=(first_stop and k == NK_X - 1),
                    )
                tiles.append(gb)
            return tiles

        g_cur = x_matmuls(0, first_stop=True)

        hT_prev = None
        for t in range(T_STEPS):
            ci = t // CHUNK_STEPS
            w = t % CHUNK_STEPS
            if w == 1 and ci + 1 < NCHUNK:
                issue_gather(ci + 1)

            # h-part matmuls close step t's accumulation (block order i,g,f,o
            # so activations can start per-block).
            if hT_prev is not None:
                for nb in range(NB):
                    for k in range(NK_H):
                        nc.tensor.matmul(
                            g_cur[nb][:],
                            lhsT=hT_prev[:, k * 128:(k + 1) * 128],
                            rhs=wh_sb[:, k, nb * NBW:(nb + 1) * NBW],
                            start=False,
                            stop=(k == NK_H - 1),
                        )

            # x-part for step t+1 runs on PE during step t's activation tail.
            if t + 1 < T_STEPS:
                g_next = x_matmuls(t + 1, first_stop=False)
            else:
                g_next = None

            if with_gate_bias:
                for nb in range(NB):
                    nc.vector.tensor_add(
                        g_cur[nb][:], g_cur[nb][:],
                        bgc[:, nb * NBW:(nb + 1) * NBW],
                    )

            i_sb = work.tile([128, U], bf16, tag="i_sb", name=f"i{t}")
            nc.scalar.activation(i_sb[:], g_cur[0][:], AF.Sigmoid)
            g_sb = work.tile([128, U], bf16, tag="g_sb", name=f"g{t}")
            nc.scalar.activation(g_sb[:], g_cur[1][:], AF.Tanh)

            if t == 0:
                # c = i*g (c starts at zero; avoids a memset feeding a TT)
                nc.vector.tensor_mul(c_sb[:], i_sb[:], g_sb[:])
                f_sb = None
            else:
                pp = work.tile([128, U], bf16, tag="pp", name=f"pp{t}")
                nc.vector.tensor_mul(pp[:], i_sb[:], g_sb[:])
            f_sb = work.tile([128, U], bf16, tag="f_sb", name=f"f{t}")
            nc.scalar.activation(f_sb[:], g_cur[2][:], AF.Sigmoid)
            if t > 0:
                fc = work.tile([128, U], f32, tag="fc", name=f"fc{t}")
                nc.vector.tensor_mul(fc[:], f_sb[:], c_sb[:])
                nc.vector.tensor_add(c_sb[:], fc[:], pp[:])
            o_sb = work.tile([128, U], bf16, tag="o_sb", name=f"o{t}")
            nc.scalar.activation(o_sb[:], g_cur[3][:], AF.Sigmoid)

            tct = work.tile([128, U], bf16, tag="tct", name=f"tct{t}")
            nc.scalar.activation(tct[:], c_sb[:], AF.Tanh)
            h = work.tile([128, U], bf16, tag="h", name=f"h{t}")
            nc.vector.tensor_mul(h[:], o_sb[:], tct[:])

            # hT for the next step: PE transpose into a borrowed psum slot.
            trp = psum.tile([128, U], bf16, tag="gi", name=f"trp{t}")
            for k in range(NK_H):
                nc.tensor.transpose(
                    trp[:, k * 128:(k + 1) * 128],
                    h[:, k * 128:(k + 1) * 128],
                    ident[:],
                )
            hT = work.tile([128, U], bf16, tag="hT", name=f"hT{t}")
            nc.vector.tensor_copy(hT[:], trp[:])
            hT_prev = hT
            g_cur = g_next

        if os.environ.get("KERNEL_STOP_AFTER", "") == "recur":
            nc.gpsimd.dma_start(out_d[:, 0:U], hT_prev[:])
        else:
            # ---- MLP head: P = h_final @ W1half -> pairwise AllReduce -> relu
            pps = psum.tile([128, 64], f32, tag="gg", name="pps")
            for k in range(NK_H):
                nc.tensor.matmul(
                    pps[:],
                    lhsT=hT_prev[:, k * 128:(k + 1) * 128],
                    rhs=w1_sb[:, k, :],
                    start=(k == 0),
                    stop=(k == NK_H - 1),
                )
            p_sb = work.tile([128, 64], f32, tag="p_sb", bufs=1)
            nc.vector.tensor_copy(p_sb[:], pps[:])
            cc_in = dram.tile([128, 64], f32, name="cc_in")
            cc_out = dram.tile([128, 64], f32, name="cc_out")
            nc.sync.dma_start(cc_in[:], p_sb[:])
            if os.environ.get("KERNEL_SKIP_CC"):
                nc.sync.dma_start(cc_out[:], cc_in[:])
            else:
                nc.gpsimd.collective_compute(
                    "AllReduce",
                    mybir.AluOpType.add,
                    replica_groups=[[0, 4], [1, 5], [2, 6], [3, 7]],
                    ins=[cc_in.opt()],
                    outs=[cc_out.opt()],
                )
            p2_sb = work.tile([128, 64], f32, tag="p2_sb", bufs=1)
            nc.sync.dma_start(p2_sb[:], cc_out[:])
            nc.vector.tensor_add(p2_sb[:], p2_sb[:], b1c[:])
            hid = work.tile([128, 64], bf16, tag="hid", bufs=1)
            nc.scalar.activation(hid[:], p2_sb[:], AF.Relu)

            hps = psum.tile([64, 128], bf16, tag="gf", name="hps")
            nc.tensor.transpose(hps[:], hid[:], ident[:])
            hidT = work.tile([64, 128], bf16, tag="hidT", bufs=1)
            nc.vector.tensor_copy(hidT[:], hps[:])

            logits = work.tile([128, V], f32, tag="logits", bufs=1)
            for vc in range(NVCH):
                lp = psum.tile([128, VCH], f32, tag=GT[vc % 4], name=f"lp{vc}")
                nc.tensor.matmul(
                    lp[:],
                    lhsT=hidT[:],
                    rhs=w2_sb[:, vc * VCH:(vc + 1) * VCH],
                    start=True,
                    stop=True,
                )
                nc.vector.tensor_copy(logits[:, vc * VCH:(vc + 1) * VCH], lp[:])
            if with_b2:
                nc.vector.tensor_add(logits[:], logits[:], b2_sb[:])

            negmax = work.tile([128, 1], f32, tag="negmax", bufs=1)
            nc.vector.reduce_max(
                negmax[:], logits[:], axis=mybir.AxisListType.X, negate=True
            )
            exps = work.tile([128, V], bf16, tag="exps", bufs=1)
            sume = work.tile([128, 1], f32, tag="sume", bufs=1)
            nc.scalar.activation(
                exps[:], logits[:], AF.Exp, bias=negmax[:], accum_out=sume[:]
            )
            rcp = work.tile([128, 1], f32, tag="rcp", bufs=1)
            nc.vector.reciprocal(rcp[:], sume[:])
            nc.vector.tensor_scalar_mul(logits[:], exps[:], rcp[:])
            nc.sync.dma_start(out_d[:], logits[:])

    nc.finalize()
    return nc


def _get_program(with_gate_bias: bool, with_b2: bool):
    key = (with_gate_bias, with_b2, T_STEPS)
    if key not in _prog_cache:
        _prog_cache[key] = _build_program(with_gate_bias, with_b2)
    return _prog_cache[key]


# gate column permutation: reference order [i f g o] -> kernel order [i g f o]
_PERM = np.concatenate(
    [np.arange(0, U), np.arange(2 * U, 3 * U), np.arange(U, 2 * U),
     np.arange(3 * U, 4 * U)]
)


def _pack_w(Wx, Wh, b):
    bf = ml_dtypes.bfloat16
    wxp = np.ascontiguousarray(
        Wx[:, _PERM].reshape(NK_X, 128, G4).astype(bf)
    )
    whp = np.ascontiguousarray(
        Wh[:, _PERM].reshape(NK_H, 128, G4).astype(bf)
    )
    bp = np.ascontiguousarray(b[_PERM].astype(np.float32))
    return wxp, whp, bp


def _make_idx(tokens_tmajor_flat):
    # dma_gather reads index i from [i % 16, i // 16]; the 16-partition index
    # block must be replicated for each of the 8 gpsimd cores (128 partitions).
    wrapped = tokens_tmajor_flat.astype(np.int16).reshape(-1, 16).T
    return np.ascontiguousarray(np.tile(wrapped, (8, 1)))


def prepare(inputs):
    """Build (nc, in_maps) for the 8 cores from full unsharded inputs."""
    bf = ml_dtypes.bfloat16
    sentence = np.asarray(inputs["sentence"])
    emb = np.asarray(inputs["emb"], np.float32)
    Wx_fw = np.asarray(inputs["Wx_fw"], np.float32)
    Wh_fw = np.asarray(inputs["Wh_fw"], np.float32)
    b_fw = np.asarray(inputs["b_fw"], np.float32)
    Wx_bw = np.asarray(inputs["Wx_bw"], np.float32)
    Wh_bw = np.asarray(inputs["Wh_bw"], np.float32)
    b_bw = np.asarray(inputs["b_bw"], np.float32)
    W1 = np.asarray(inputs["W1"], np.float32)
    b1 = np.asarray(inputs["b1"], np.float32)
    W2 = np.asarray(inputs["W2"], np.float32)
    b2 = np.asarray(inputs["b2"], np.float32)

    with_gate_bias = bool(np.any(b_fw) or np.any(b_bw))
    with_b2 = bool(np.any(b2))
    nc = _get_program(with_gate_bias, with_b2)

    emb16 = np.ascontiguousarray(emb.astype(bf))
    wx_f, wh_f, bg_f = _pack_w(Wx_fw, Wh_fw, b_fw)
    wx_b, wh_b, bg_b = _pack_w(Wx_bw, Wh_bw, b_bw)
    w1f = np.ascontiguousarray(W1[0:U].reshape(NK_H, 128, 64).astype(bf))
    w1b = np.ascontiguousarray(W1[U:2 * U].reshape(NK_H, 128, 64).astype(bf))
    w2p = np.ascontiguousarray(W2.astype(bf))
    b1bc = np.ascontiguousarray(np.broadcast_to(b1[None, :], (128, 64)).astype(np.float32))

    in_maps = []
    for c in range(NCORES):
        fw = c < 4
        rows = slice(128 * (c % 4), 128 * (c % 4) + 128)
        toks = sentence[rows][:, :T]
        if not fw:
            toks = toks[:, ::-1]
        flat = np.ascontiguousarray(toks.T).reshape(-1)  # t-major
        m = {
            "emb16": emb16,
            "idx16": _make_idx(flat),
            "wx": wx_f if fw else wx_b,
            "wh": wh_f if fw else wh_b,
            "w1h": w1f if fw else w1b,
            "w2": w2p,
            "b1bc": b1bc,
        }
        if with_gate_bias:
            bg = bg_f if fw else bg_b
            m["bgbc"] = np.ascontiguousarray(
                np.broadcast_to(bg[None, :], (128, G4)).astype(np.float32)
            )
        if with_b2:
            m["b2bc"] = np.ascontiguousarray(
                np.broadcast_to(b2[None, :], (128, V)).astype(np.float32)
            )
        in_maps.append(m)
    return nc, in_maps


def kernel(**inputs):
    from concourse.bass_utils import run_bass_kernel_spmd

    nc, in_maps = prepare(inputs)
    res = run_bass_kernel_spmd(
        nc, in_maps, core_ids=list(range(NCORES)),
        trace=bool(int(os.environ.get("KERNEL_TRACE", "0"))),
    )
    out = np.concatenate([res.results[c]["out"] for c in range(4)], axis=0)
    kernel.last_results = res
    return out.astype(np.float32)


# revision 5
# speedup vs baseline: 44.8258x; 1.2113x over previous
# Bidirectional LSTM (B=512, T=256, E=256, U=512) + MLP + softmax(V=10000)
# on 8 trn2 NeuronCores.
#
# Distribution: data-parallel over batch x direction. Cores 0-3 run the
# forward LSTM on batch slices of 128; cores 4-7 run the backward LSTM on the
# same slices (time-reversed token stream via the gather index table, so the
# SPMD program is identical on every core). Core pairs (i, i+4) AllReduce
# their partial h @ W1-half products; the host keeps the fw copies.
#
# v2 pipeline (per step, per core, batch 128):
#   gates live in 4 per-block PSUM tiles [128,512] f32 (blocks = i,g,f,o in
#   permuted column order), each tag double-buffered: the x-part matmuls for
#   step t+1 accumulate into the other slot DURING step t's activation tail,
#   so TensorE never idles and HAM stays un-throttled.
#   Per-block activations ([i] sigmoid -> [g] tanh -> [f] sigmoid -> [o]
#   sigmoid) start as soon as that block's h-matmuls finish, overlapping the
#   remaining h/x matmuls.  c = f*c + i*g (fp32, DVE); h = o*tanh(c) (bf16);
#   hT via 4 PE transposes into a borrowed PSUM slot + DVE copy.
import os
import numpy as np
import ml_dtypes

B, T, E, U, V = 512, 256, 256, 512, 10000
G4 = 4 * U
NCORES = 8
BC = 128              # batch rows per core
NK_X = E // 128       # 2 contraction tiles for x
NK_H = U // 128       # 4 contraction tiles for h
NBW = 512             # matmul n-block width (= one PSUM bank fp32)
NB = G4 // NBW        # 4 n-blocks: [i g f o] in permuted column order
TOK = BC * T          # 32768 tokens gathered per core
T_STEPS = int(os.environ.get("KERNEL_T", T))
CHUNK_STEPS = 4   # 512 tokens per dma_gather (>512 idxs crashes SWDGE)
CHUNK_TOK = BC * CHUNK_STEPS
NCHUNK = (T_STEPS + CHUNK_STEPS - 1) // CHUNK_STEPS
VCH = 500             # logits chunk width
NVCH = V // VCH

_prog_cache = {}


def _build_program(with_gate_bias: bool, with_b2: bool):
    import concourse.bass as bass
    import concourse.mybir as mybir
    import concourse.tile as tile
    from concourse import bacc
    from concourse.masks import make_identity
    from contextlib import ExitStack

    f32 = mybir.dt.float32
    bf16 = mybir.dt.bfloat16
    i16 = mybir.dt.int16
    AF = mybir.ActivationFunctionType

    nc = bacc.Bacc("TRN2", debug=False, enable_asserts=False, num_devices=NCORES)

    emb_d = nc.dram_tensor("emb16", [V, E], bf16, kind="ExternalInput").ap()
    idx_d = nc.dram_tensor("idx16", [128, TOK // 16], i16, kind="ExternalInput").ap()
    wx_d = nc.dram_tensor("wx", [NK_X, 128, G4], bf16, kind="ExternalInput").ap()
    wh_d = nc.dram_tensor("wh", [NK_H, 128, G4], bf16, kind="ExternalInput").ap()
    w1_d = nc.dram_tensor("w1h", [NK_H, 128, 64], bf16, kind="ExternalInput").ap()
    w2_d = nc.dram_tensor("w2", [64, V], bf16, kind="ExternalInput").ap()
    b1_d = nc.dram_tensor("b1bc", [128, 64], f32, kind="ExternalInput").ap()
    if with_gate_bias:
        bg_d = nc.dram_tensor("bgbc", [128, G4], f32, kind="ExternalInput").ap()
    if with_b2:
        b2_d = nc.dram_tensor("b2bc", [128, V], f32, kind="ExternalInput").ap()
    out_d = nc.dram_tensor("out", [BC, V], f32, kind="ExternalOutput").ap()

    with tile.TileContext(nc) as tc, ExitStack() as ctx:
        const = ctx.enter_context(tc.tile_pool(name="const", bufs=1))
        gpool = ctx.enter_context(tc.tile_pool(name="gather", bufs=3))
        work = ctx.enter_context(tc.tile_pool(name="work", bufs=2))
        psum = ctx.enter_context(tc.tile_pool(name="psum", bufs=2, space="PSUM"))
        dram = ctx.enter_context(tc.tile_pool(name="dram", bufs=1, space="DRAM"))

        wx_sb = const.tile([128, NK_X, G4], bf16)
        for k in range(NK_X):
            nc.sync.dma_start(wx_sb[:, k, :], wx_d[k])
        wh_sb = const.tile([128, NK_H, G4], bf16)
        for k in range(NK_H):
            nc.sync.dma_start(wh_sb[:, k, :], wh_d[k])
        w1_sb = const.tile([128, NK_H, 64], bf16)
        for k in range(NK_H):
            nc.sync.dma_start(w1_sb[:, k, :], w1_d[k])
        w2_sb = const.tile([64, V], bf16)
        nc.sync.dma_start(w2_sb[:], w2_d[:])
        b1_sb = const.tile([128, 64], f32)
        nc.sync.dma_start(b1_sb[:], b1_d[:])
        # DVE pre-copy so downstream tensor_tensor ops have a same-engine dep
        # (walrus TT format has a single sync-wait slot).
        b1c = const.tile([128, 64], f32)
        nc.vector.tensor_copy(b1c[:], b1_sb[:])
        if with_gate_bias:
            bg_sb = const.tile([128, G4], f32)
            nc.sync.dma_start(bg_sb[:], bg_d[:])
            bgc = const.tile([128, G4], f32)
            nc.vector.tensor_copy(bgc[:], bg_sb[:])
        if with_b2:
            b2_sb = const.tile([128, V], f32)
            nc.sync.dma_start(b2_sb[:], b2_d[:])
        idx_sb = const.tile([128, TOK // 16], i16)
        nc.sync.dma_start(idx_sb[:], idx_d[:])
        ident = const.tile([128, 128], bf16)
        make_identity(nc, ident[:])
        c_sb = const.tile([128, U], bf16)

        xg_tiles = {}

        def issue_gather(ci):
            xg = gpool.tile(
                [128, NK_X, CHUNK_TOK], bf16, tag="xg", name=f"xg{ci}"
            )
            nc.gpsimd.dma_gather(
                xg[:],
                emb_d[:],
                idx_sb[:, ci * (CHUNK_TOK // 16):(ci + 1) * (CHUNK_TOK // 16)],
                CHUNK_TOK,
                CHUNK_TOK,
                E,
                transpose=True,
            )
            xg_tiles[ci] = xg

        issue_gather(0)

        GT = ["gi", "gg", "gf", "go"]

        def x_matmuls(t, blocks, first_stop):
            # x-part matmuls for step t into fresh per-block psum tiles.
            # first_stop: close the group (t==0 only; otherwise h matmuls
            # continue the accumulation at step t).
            ci = t // CHUNK_STEPS
            w = t % CHUNK_STEPS
            xg = xg_tiles[ci]
            tiles = []
            for nb in blocks:
                gb = psum.tile([128, NBW], f32, tag=GT[nb], name=f"{GT[nb]}_{t}")
                for k in range(NK_X):
                    nc.tensor.matmul(
                        gb[:],
                        lhsT=xg[:, k, w * BC:(w + 1) * BC],
                        rhs=wx_sb[:, k, nb * NBW:(nb + 1) * NBW],
                        start=(k == 0),
                        stop=(first_stop and k == NK_X - 1),
                    )
                tiles.append(gb)
            return tiles

        g_cur = x_matmuls(0, range(NB), first_stop=True)

        hT_prev = None
        for t in range(T_STEPS):
            ci = t // CHUNK_STEPS
            w = t % CHUNK_STEPS
            if w == 1 and ci + 1 < NCHUNK:
                issue_gather(ci + 1)

            # h-part matmuls close step t's accumulation (block order i,g,f,o
            # so activations can start per-block).
            if hT_prev is not None:
                for nb in range(NB):
                    for k in range(NK_H):
                        nc.tensor.matmul(
                            g_cur[nb][:],
                            lhsT=hT_prev[:, k * 128:(k + 1) * 128],
                            rhs=wh_sb[:, k, nb * NBW:(nb + 1) * NBW],
                            start=False,
                            stop=(k == NK_H - 1),
                        )

            # x-part for step t+1, blocks i,g: fills PE while step t's first
            # activations run.  Blocks f,o are emitted after the transposes so
            # the PE queue reaches trp_k as soon as h_k lands.
            if t + 1 < T_STEPS:
                g_next = x_matmuls(t + 1, (0, 1), first_stop=False)
            else:
                g_next = None

            if with_gate_bias:
                for nb in range(NB):
                    nc.vector.tensor_add(
                        g_cur[nb][:], g_cur[nb][:],
                        bgc[:, nb * NBW:(nb + 1) * NBW],
                    )

            i_sb = work.tile([128, U], bf16, tag="i_sb", name=f"i{t}")
            nc.scalar.activation(i_sb[:], g_cur[0][:], AF.Sigmoid)
            g_sb = work.tile([128, U], bf16, tag="g_sb", name=f"g{t}")
            nc.scalar.activation(g_sb[:], g_cur[1][:], AF.Tanh)

            if t == 0:
                # c = i*g (c starts at zero; avoids a memset feeding a TT)
                nc.vector.tensor_mul(c_sb[:], i_sb[:], g_sb[:])
            else:
                pp = work.tile([128, U], bf16, tag="pp", name=f"pp{t}")
                nc.vector.tensor_mul(pp[:], i_sb[:], g_sb[:])
            f_sb = work.tile([128, U], bf16, tag="f_sb", name=f"f{t}")
            nc.scalar.activation(f_sb[:], g_cur[2][:], AF.Sigmoid)
            o_sb = work.tile([128, U], bf16, tag="o_sb", name=f"o{t}")
            nc.scalar.activation(o_sb[:], g_cur[3][:], AF.Sigmoid)

            # k-sliced tail: each 128-col slice flows c->tanh->h->transpose->
            # copy independently so next step's h-matmuls (k-inner) can start
            # as soon as slice 0 lands in SBUF.
            if t > 0:
                fc = work.tile([128, U], bf16, tag="fc", name=f"fc{t}")
            tct = work.tile([128, U], bf16, tag="tct", name=f"tct{t}")
            h = work.tile([128, U], bf16, tag="h", name=f"h{t}")
            trp = psum.tile([128, U], bf16, tag="gi", name=f"trp{t}")
            hT = work.tile([128, U], bf16, tag="hT", name=f"hT{t}")
            for k in range(NK_H):
                s = slice(k * 128, (k + 1) * 128)
                if t > 0:
                    nc.vector.tensor_mul(fc[:, s], f_sb[:, s], c_sb[:, s])
                    nc.vector.tensor_add(c_sb[:, s], fc[:, s], pp[:, s])
                nc.scalar.activation(tct[:, s], c_sb[:, s], AF.Tanh)
                nc.vector.tensor_mul(h[:, s], o_sb[:, s], tct[:, s])
                nc.tensor.transpose(trp[:, s], h[:, s], ident[:])
                nc.vector.tensor_copy(hT[:, s], trp[:, s])

            # x-part for step t+1, blocks f,o (behind the transposes on PE).
            if g_next is not None:
                g_next.extend(x_matmuls(t + 1, (2, 3), first_stop=False))

            hT_prev = hT
            g_cur = g_next

        if os.environ.get("KERNEL_STOP_AFTER", "") == "recur":
            nc.gpsimd.dma_start(out_d[:, 0:U], hT_prev[:])
        else:
            # ---- MLP head: P = h_final @ W1half -> pairwise AllReduce -> relu
            pps = psum.tile([128, 64], f32, tag="gg", name="pps")
            for k in range(NK_H):
                nc.tensor.matmul(
                    pps[:],
                    lhsT=hT_prev[:, k * 128:(k + 1) * 128],
                    rhs=w1_sb[:, k, :],
                    start=(k == 0),
                    stop=(k == NK_H - 1),
                )
            p_sb = work.tile([128, 64], f32, tag="p_sb", bufs=1)
            nc.vector.tensor_copy(p_sb[:], pps[:])
            cc_in = dram.tile([128, 64], f32, name="cc_in")
            cc_out = dram.tile([128, 64], f32, name="cc_out")
            nc.sync.dma_start(cc_in[:], p_sb[:])
            if os.environ.get("KERNEL_SKIP_CC"):
                nc.sync.dma_start(cc_out[:], cc_in[:])
            else:
                nc.gpsimd.collective_compute(
                    "AllReduce",
                    mybir.AluOpType.add,
                    replica_groups=[[0, 4], [1, 5], [2, 6], [3, 7]],
                    ins=[cc_in.opt()],
                    outs=[cc_out.opt()],
                )
            p2_sb = work.tile([128, 64], f32, tag="p2_sb", bufs=1)
            nc.sync.dma_start(p2_sb[:], cc_out[:])
            nc.vector.tensor_add(p2_sb[:], p2_sb[:], b1c[:])
            hid = work.tile([128, 64], bf16, tag="hid", bufs=1)
            nc.scalar.activation(hid[:], p2_sb[:], AF.Relu)

            hps = psum.tile([64, 128], bf16, tag="gf", name="hps")
            nc.tensor.transpose(hps[:], hid[:], ident[:])
            hidT = work.tile([64, 128], bf16, tag="hidT", bufs=1)
            nc.vector.tensor_copy(hidT[:], hps[:])

            logits = work.tile([128, V], f32, tag="logits", bufs=1)
            for vc in range(NVCH):
                lp = psum.tile([128, VCH], f32, tag=GT[vc % 4], name=f"lp{vc}")
                nc.tensor.matmul(
                    lp[:],
                    lhsT=hidT[:],
                    rhs=w2_sb[:, vc * VCH:(vc + 1) * VCH],
                    start=True,
                    stop=True,
                )
                nc.vector.tensor_copy(logits[:, vc * VCH:(vc + 1) * VCH], lp[:])
            if with_b2:
                nc.vector.tensor_add(logits[:], logits[:], b2_sb[:])

            negmax = work.tile([128, 1], f32, tag="negmax", bufs=1)
            nc.vector.reduce_max(
                negmax[:], logits[:], axis=mybir.AxisListType.X, negate=True
            )
            exps = work.tile([128, V], bf16, tag="exps", bufs=1)
            sume = work.tile([128, 1], f32, tag="sume", bufs=1)
            nc.scalar.activation(
                exps[:], logits[:], AF.Exp, bias=negmax[:], accum_out=sume[:]
            )
            rcp = work.tile([128, 1], f32, tag="rcp", bufs=1)
            nc.vector.reciprocal(rcp[:], sume[:])
            nc.vector.tensor_scalar_mul(logits[:], exps[:], rcp[:])
            nc.sync.dma_start(out_d[:], logits[:])

    nc.finalize()
    return nc


def _get_program(with_gate_bias: bool, with_b2: bool):
    key = (with_gate_bias, with_b2, T_STEPS)
    if key not in _prog_cache:
        _prog_cache[key] = _build_program(with_gate_bias, with_b2)
    return _prog_cache[key]


# gate column permutation: reference order [i f g o] -> kernel order [i g f o]
_PERM = np.concatenate(
    [np.arange(0, U), np.arange(2 * U, 3 * U), np.arange(U, 2 * U),
     np.arange(3 * U, 4 * U)]
)


def _pack_w(Wx, Wh, b):
    bf = ml_dtypes.bfloat16
    wxp = np.ascontiguousarray(
        Wx[:, _PERM].reshape(NK_X, 128, G4).astype(bf)
    )
    whp = np.ascontiguousarray(
        Wh[:, _PERM].reshape(NK_H, 128, G4).astype(bf)
    )
    bp = np.ascontiguousarray(b[_PERM].astype(np.float32))
    return wxp, whp, bp


def _make_idx(tokens_tmajor_flat):
    # dma_gather reads index i from [i % 16, i // 16]; the 16-partition index
    # block must be replicated for each of the 8 gpsimd cores (128 partitions).
    wrapped = tokens_tmajor_flat.astype(np.int16).reshape(-1, 16).T
    return np.ascontiguousarray(np.tile(wrapped, (8, 1)))


def prepare(inputs):
    """Build (nc, in_maps) for the 8 cores from full unsharded inputs."""
    bf = ml_dtypes.bfloat16
    sentence = np.asarray(inputs["sentence"])
    emb = np.asarray(inputs["emb"], np.float32)
    Wx_fw = np.asarray(inputs["Wx_fw"], np.float32)
    Wh_fw = np.asarray(inputs["Wh_fw"], np.float32)
    b_fw = np.asarray(inputs["b_fw"], np.float32)
    Wx_bw = np.asarray(inputs["Wx_bw"], np.float32)
    Wh_bw = np.asarray(inputs["Wh_bw"], np.float32)
    b_bw = np.asarray(inputs["b_bw"], np.float32)
    W1 = np.asarray(inputs["W1"], np.float32)
    b1 = np.asarray(inputs["b1"], np.float32)
    W2 = np.asarray(inputs["W2"], np.float32)
    b2 = np.asarray(inputs["b2"], np.float32)

    with_gate_bias = bool(np.any(b_fw) or np.any(b_bw))
    with_b2 = bool(np.any(b2))
    nc = _get_program(with_gate_bias, with_b2)

    emb16 = np.ascontiguousarray(emb.astype(bf))
    wx_f, wh_f, bg_f = _pack_w(Wx_fw, Wh_fw, b_fw)
    wx_b, wh_b, bg_b = _pack_w(Wx_bw, Wh_bw, b_bw)
    w1f = np.ascontiguousarray(W1[0:U].reshape(NK_H, 128, 64).astype(bf))
    w1b = np.ascontiguousarray(W1[U:2 * U].reshape(NK_H, 128, 64).astype(bf))
    w2p = np.ascontiguousarray(W2.astype(bf))
    b1bc = np.ascontiguousarray(np.broadcast_to(b1[None, :], (128, 64)).astype(np.float32))

    in_maps = []
    for c in range(NCORES):
        fw = c < 4
        rows = slice(128 * (c % 4), 128 * (c % 4) + 128)
        toks = sentence[rows][:, :T]
        if not fw:
            toks = toks[:, ::-1]
        flat = np.ascontiguousarray(toks.T).reshape(-1)  # t-major
        m = {
            "emb16": emb16,
            "idx16": _make_idx(flat),
            "wx": wx_f if fw else wx_b,
            "wh": wh_f if fw else wh_b,
            "w1h": w1f if fw else w1b,
            "w2": w2p,
            "b1bc": b1bc,
        }
        if with_gate_bias:
            bg = bg_f if fw else bg_b
            m["bgbc"] = np.ascontiguousarray(
                np.broadcast_to(bg[None, :], (128, G4)).astype(np.float32)
            )
        if with_b2:
            m["b2bc"] = np.ascontiguousarray(
                np.broadcast_to(b2[None, :], (128, V)).astype(np.float32)
            )
        in_maps.append(m)
    return nc, in_maps


def kernel(**inputs):
    from concourse.bass_utils import run_bass_kernel_spmd

    nc, in_maps = prepare(inputs)
    res = run_bass_kernel_spmd(
        nc, in_maps, core_ids=list(range(NCORES)),
        trace=bool(int(os.environ.get("KERNEL_TRACE", "0"))),
    )
    out = np.concatenate([res.results[c]["out"] for c in range(4)], axis=0)
    kernel.last_results = res
    return out.astype(np.float32)


# revision 20
# speedup vs baseline: 45.1952x; 1.0082x over previous
# Bidirectional LSTM (B=512, T=256, E=256, U=512) + MLP + softmax(V=10000)
# on 8 trn2 NeuronCores.
#
# Distribution: data-parallel over batch x direction. Cores 0-3 run the
# forward LSTM on batch slices of 128; cores 4-7 run the backward LSTM on the
# same slices (time-reversed token stream via the gather index table, so the
# SPMD program is identical on every core). Core pairs (i, i+4) AllReduce
# their partial h @ W1-half products; the host keeps the fw copies.
#
# v2 pipeline (per step, per core, batch 128):
#   gates live in 4 per-block PSUM tiles [128,512] f32 (blocks = i,g,f,o in
#   permuted column order), each tag double-buffered: the x-part matmuls for
#   step t+1 accumulate into the other slot DURING step t's activation tail,
#   so TensorE never idles and HAM stays un-throttled.
#   Per-block activations ([i] sigmoid -> [g] tanh -> [f] sigmoid -> [o]
#   sigmoid) start as soon as that block's h-matmuls finish, overlapping the
#   remaining h/x matmuls.  c = f*c + i*g (fp32, DVE); h = o*tanh(c) (bf16);
#   hT via 4 PE transposes into a borrowed PSUM slot + DVE copy.
import os
import numpy as np
import ml_dtypes

B, T, E, U, V = 512, 256, 256, 512, 10000
G4 = 4 * U
NCORES = 8
BC = 128              # batch rows per core
NK_X = E // 128       # 2 contraction tiles for x
NK_H = U // 128       # 4 contraction tiles for h
NBW = 512             # matmul n-block width (= one PSUM bank fp32)
NB = G4 // NBW        # 4 n-blocks: [i g f o] in permuted column order
TOK = BC * T          # 32768 tokens gathered per core
T_STEPS = int(os.environ.get("KERNEL_T", T))
CHUNK_STEPS = 4   # 512 tokens per dma_gather (>512 idxs crashes SWDGE)
CHUNK_TOK = BC * CHUNK_STEPS
NCHUNK = (T_STEPS + CHUNK_STEPS - 1) // CHUNK_STEPS
VCH = 500             # logits chunk width
NVCH = V // VCH

_prog_cache = {}


def _build_program(with_gate_bias: bool, with_b2: bool):
    import concourse.bass as bass
    import concourse.mybir as mybir
    import concourse.tile as tile
    from concourse import bacc
    from concourse.masks import make_identity
    from contextlib import ExitStack

    f32 = mybir.dt.float32
    bf16 = mybir.dt.bfloat16
    i16 = mybir.dt.int16
    AF = mybir.ActivationFunctionType

    nc = bacc.Bacc("TRN2", debug=False, enable_asserts=False, num_devices=NCORES)

    emb_d = nc.dram_tensor("emb16", [V, E], bf16, kind="ExternalInput").ap()
    idx_d = nc.dram_tensor("idx16", [128, TOK // 16], i16, kind="ExternalInput").ap()
    wx_d = nc.dram_tensor("wx", [NK_X, 128, G4], bf16, kind="ExternalInput").ap()
    wh_d = nc.dram_tensor("wh", [NK_H, 128, G4], bf16, kind="ExternalInput").ap()
    w1_d = nc.dram_tensor("w1h", [NK_H, 128, 64], bf16, kind="ExternalInput").ap()
    w2_d = nc.dram_tensor("w2", [64, V], bf16, kind="ExternalInput").ap()
    b1_d = nc.dram_tensor("b1bc", [128, 64], f32, kind="ExternalInput").ap()
    if with_gate_bias:
        bg_d = nc.dram_tensor("bgbc", [128, G4], f32, kind="ExternalInput").ap()
    if with_b2:
        b2_d = nc.dram_tensor("b2bc", [128, V], f32, kind="ExternalInput").ap()
    out_d = nc.dram_tensor("out", [BC, V], f32, kind="ExternalOutput").ap()

    with tile.TileContext(nc) as tc, ExitStack() as ctx:
        const = ctx.enter_context(tc.tile_pool(name="const", bufs=1))
        gpool = ctx.enter_context(tc.tile_pool(name="gather", bufs=3))
        work = ctx.enter_context(tc.tile_pool(name="work", bufs=2))
        psum = ctx.enter_context(tc.tile_pool(name="psum", bufs=2, space="PSUM"))
        dram = ctx.enter_context(tc.tile_pool(name="dram", bufs=1, space="DRAM"))

        # idx first: the first gather (and so step 0) depends only on it.
        idx_sb = const.tile([128, TOK // 16], i16)
        nc.sync.dma_start(idx_sb[:], idx_d[:])
        wx_sb = const.tile([128, NK_X, G4], bf16)
        for k in range(NK_X):
            nc.sync.dma_start(wx_sb[:, k, :], wx_d[k])
        wh_sb = const.tile([128, NK_H, G4], bf16)
        for k in range(NK_H):
            nc.sync.dma_start(wh_sb[:, k, :], wh_d[k])
        w1_sb = const.tile([128, NK_H, 64], bf16)
        for k in range(NK_H):
            nc.sync.dma_start(w1_sb[:, k, :], w1_d[k])
        w2_sb = const.tile([64, V], bf16)
        nc.sync.dma_start(w2_sb[:], w2_d[:])
        b1_sb = const.tile([128, 64], f32)
        nc.sync.dma_start(b1_sb[:], b1_d[:])
        # DVE pre-copy so downstream tensor_tensor ops have a same-engine dep
        # (walrus TT format has a single sync-wait slot).
        b1c = const.tile([128, 64], f32)
        nc.vector.tensor_copy(b1c[:], b1_sb[:])
        if with_gate_bias:
            bg_sb = const.tile([128, G4], f32)
            nc.sync.dma_start(bg_sb[:], bg_d[:])
            bgc = const.tile([128, G4], f32)
            nc.vector.tensor_copy(bgc[:], bg_sb[:])
        if with_b2:
            b2_sb = const.tile([128, V], f32)
            nc.sync.dma_start(b2_sb[:], b2_d[:])
        ident = const.tile([128, 128], bf16)
        make_identity(nc, ident[:])
        c_sb = const.tile([128, U], bf16)

        xg_tiles = {}

        def issue_gather(ci):
            xg = gpool.tile(
                [128, NK_X, CHUNK_TOK], bf16, tag="xg", name=f"xg{ci}"
            )
            nc.gpsimd.dma_gather(
                xg[:],
                emb_d[:],
                idx_sb[:, ci * (CHUNK_TOK // 16):(ci + 1) * (CHUNK_TOK // 16)],
                CHUNK_TOK,
                CHUNK_TOK,
                E,
                transpose=True,
            )
            xg_tiles[ci] = xg

        issue_gather(0)

        GT = ["gi", "gg", "gf", "go"]

        def x_matmuls(t, blocks, first_stop):
            # x-part matmuls for step t into fresh per-block psum tiles.
            # first_stop: close the group (t==0 only; otherwise h matmuls
            # continue the accumulation at step t).
            ci = t // CHUNK_STEPS
            w = t % CHUNK_STEPS
            xg = xg_tiles[ci]
            tiles = []
            for nb in blocks:
                gb = psum.tile([128, NBW], f32, tag=GT[nb], name=f"{GT[nb]}_{t}")
                for k in range(NK_X):
                    nc.tensor.matmul(
                        gb[:],
                        lhsT=xg[:, k, w * BC:(w + 1) * BC],
                        rhs=wx_sb[:, k, nb * NBW:(nb + 1) * NBW],
                        start=(k == 0),
                        stop=(first_stop and k == NK_X - 1),
                    )
                tiles.append(gb)
            return tiles

        g_cur = x_matmuls(0, range(NB), first_stop=True)

        hT_prev = None
        for t in range(T_STEPS):
            ci = t // CHUNK_STEPS
            w = t % CHUNK_STEPS
            if w == 1 and ci + 1 < NCHUNK:
                issue_gather(ci + 1)

            # h-part matmuls close step t's accumulation (block order i,g,f,o
            # so activations can start per-block).
            if hT_prev is not None:
                for nb in range(NB):
                    for k in range(NK_H):
                        nc.tensor.matmul(
                            g_cur[nb][:],
                            lhsT=hT_prev[:, k * 128:(k + 1) * 128],
                            rhs=wh_sb[:, k, nb * NBW:(nb + 1) * NBW],
                            start=False,
                            stop=(k == NK_H - 1),
                        )

            # x-part for step t+1, blocks i,g: fills PE while step t's first
            # activations run.  Blocks f,o are emitted after the transposes so
            # the PE queue reaches trp_k as soon as h_k lands.
            if t + 1 < T_STEPS:
                g_next = x_matmuls(t + 1, (0, 1), first_stop=False)
            else:
                g_next = None

            if with_gate_bias:
                for nb in range(NB):
                    nc.vector.tensor_add(
                        g_cur[nb][:], g_cur[nb][:],
                        bgc[:, nb * NBW:(nb + 1) * NBW],
                    )

            i_sb = work.tile([128, U], bf16, tag="i_sb", name=f"i{t}")
            nc.scalar.activation(i_sb[:], g_cur[0][:], AF.Sigmoid)
            g_sb = work.tile([128, U], bf16, tag="g_sb", name=f"g{t}")
            nc.scalar.activation(g_sb[:], g_cur[1][:], AF.Tanh)

            if t == 0:
                # c = i*g (c starts at zero; avoids a memset feeding a TT)
                nc.vector.tensor_mul(c_sb[:], i_sb[:], g_sb[:])
            else:
                pp = work.tile([128, U], bf16, tag="pp", name=f"pp{t}")
                nc.vector.tensor_mul(pp[:], i_sb[:], g_sb[:])
            f_sb = work.tile([128, U], bf16, tag="f_sb", name=f"f{t}")
            nc.scalar.activation(f_sb[:], g_cur[2][:], AF.Sigmoid)
            o_sb = work.tile([128, U], bf16, tag="o_sb", name=f"o{t}")
            nc.scalar.activation(o_sb[:], g_cur[3][:], AF.Sigmoid)

            # k-sliced tail: each 128-col slice flows c->tanh->h->transpose->
            # copy independently so next step's h-matmuls (k-inner) can start
            # as soon as slice 0 lands in SBUF.
            if t > 0:
                fc = work.tile([128, U], bf16, tag="fc", name=f"fc{t}")
            tct = work.tile([128, U], bf16, tag="tct", name=f"tct{t}")
            h = work.tile([128, U], bf16, tag="h", name=f"h{t}")
            trp = psum.tile([128, U], bf16, tag="gi", name=f"trp{t}")
            hT = work.tile([128, U], bf16, tag="hT", name=f"hT{t}")
            for k in range(NK_H):
                s = slice(k * 128, (k + 1) * 128)
                if t > 0:
                    nc.vector.tensor_mul(fc[:, s], f_sb[:, s], c_sb[:, s])
                    nc.vector.tensor_add(c_sb[:, s], fc[:, s], pp[:, s])
                nc.scalar.activation(tct[:, s], c_sb[:, s], AF.Tanh)
                nc.vector.tensor_mul(h[:, s], o_sb[:, s], tct[:, s])
                nc.tensor.transpose(trp[:, s], h[:, s], ident[:])
                nc.vector.tensor_copy(hT[:, s], trp[:, s])

            # x-part for step t+1, blocks f,o (behind the transposes on PE).
            if g_next is not None:
                g_next.extend(x_matmuls(t + 1, (2, 3), first_stop=False))

            hT_prev = hT
            g_cur = g_next

        if os.environ.get("KERNEL_STOP_AFTER", "") == "recur":
            nc.gpsimd.dma_start(out_d[:, 0:U], hT_prev[:])
        else:
            # ---- MLP head: P = h_final @ W1half -> pairwise AllReduce -> relu
            pps = psum.tile([128, 64], f32, tag="gg", name="pps")
            for k in range(NK_H):
                nc.tensor.matmul(
                    pps[:],
                    lhsT=hT_prev[:, k * 128:(k + 1) * 128],
                    rhs=w1_sb[:, k, :],
                    start=(k == 0),
                    stop=(k == NK_H - 1),
                )
            p_sb = work.tile([128, 64], f32, tag="p_sb", bufs=1)
            nc.vector.tensor_copy(p_sb[:], pps[:])
            cc_in = dram.tile([128, 64], f32, name="cc_in")
            cc_out = dram.tile([128, 64], f32, name="cc_out")
            nc.sync.dma_start(cc_in[:], p_sb[:])
            if os.environ.get("KERNEL_SKIP_CC"):
                nc.sync.dma_start(cc_out[:], cc_in[:])
            else:
                nc.gpsimd.collective_compute(
                    "AllReduce",
                    mybir.AluOpType.add,
                    replica_groups=[[0, 4], [1, 5], [2, 6], [3, 7]],
                    ins=[cc_in.opt()],
                    outs=[cc_out.opt()],
                )
            p2_sb = work.tile([128, 64], f32, tag="p2_sb", bufs=1)
            nc.sync.dma_start(p2_sb[:], cc_out[:])
            nc.vector.tensor_add(p2_sb[:], p2_sb[:], b1c[:])
            hid = work.tile([128, 64], bf16, tag="hid", bufs=1)
            nc.scalar.activation(hid[:], p2_sb[:], AF.Relu)

            hps = psum.tile([64, 128], bf16, tag="gf", name="hps")
            nc.tensor.transpose(hps[:], hid[:], ident[:])
            hidT = work.tile([64, 128], bf16, tag="hidT", bufs=1)
            nc.vector.tensor_copy(hidT[:], hps[:])

            logits = work.tile([128, V], f32, tag="logits", bufs=1)
            for vc in range(NVCH):
                lp = psum.tile([128, VCH], f32, tag=GT[vc % 4], name=f"lp{vc}")
                nc.tensor.matmul(
                    lp[:],
                    lhsT=hidT[:],
                    rhs=w2_sb[:, vc * VCH:(vc + 1) * VCH],
                    start=True,
                    stop=True,
                )
                nc.vector.tensor_copy(logits[:, vc * VCH:(vc + 1) * VCH], lp[:])
            if with_b2:
                nc.vector.tensor_add(logits[:], logits[:], b2_sb[:])

            # softmax without the max-subtraction: logits here are hid @ W2
            # with |logit| < ~1 for any plausible input scale (W2 ~ 1/8,
            # hid bounded by relu of unit-scale products), so exp() cannot
            # overflow fp32.  exp/sum/scale/store pipelined in V-chunks.
            HCH = 2500
            NHC = V // HCH
            exps = work.tile([128, V], bf16, tag="exps", bufs=1)
            psum_s = work.tile([128, NHC], f32, tag="sume", bufs=1)
            for hc in range(NHC):
                s = slice(hc * HCH, (hc + 1) * HCH)
                nc.scalar.activation(
                    exps[:, s], logits[:, s], AF.Exp,
                    accum_out=psum_s[:, hc:hc + 1],
                )
            sume = work.tile([128, 1], f32, tag="sumt", bufs=1)
            nc.vector.tensor_reduce(
                sume[:], psum_s[:], op=mybir.AluOpType.add,
                axis=mybir.AxisListType.X,
            )
            rcp = work.tile([128, 1], f32, tag="rcp", bufs=1)
            nc.vector.reciprocal(rcp[:], sume[:])
            for hc in range(NHC):
                s = slice(hc * HCH, (hc + 1) * HCH)
                nc.vector.tensor_scalar_mul(logits[:, s], exps[:, s], rcp[:])
                nc.sync.dma_start(out_d[:, s], logits[:, s])

    nc.finalize()
    return nc


def _get_program(with_gate_bias: bool, with_b2: bool):
    key = (with_gate_bias, with_b2, T_STEPS)
    if key not in _prog_cache:
        _prog_cache[key] = _build_program(with_gate_bias, with_b2)
    return _prog_cache[key]


# gate column permutation: reference order [i f g o] -> kernel order [i g f o]
_PERM = np.concatenate(
    [np.arange(0, U), np.arange(2 * U, 3 * U), np.arange(U, 2 * U),
     np.arange(3 * U, 4 * U)]
)


def _pack_w(Wx, Wh, b):
    bf = ml_dtypes.bfloat16
    wxp = np.ascontiguousarray(
        Wx[:, _PERM].reshape(NK_X, 128, G4).astype(bf)
    )
    whp = np.ascontiguousarray(
        Wh[:, _PERM].reshape(NK_H, 128, G4).astype(bf)
    )
    bp = np.ascontiguousarray(b[_PERM].astype(np.float32))
    return wxp, whp, bp


def _make_idx(tokens_tmajor_flat):
    # dma_gather reads index i from [i % 16, i // 16]; the 16-partition index
    # block must be replicated for each of the 8 gpsimd cores (128 partitions).
    wrapped = tokens_tmajor_flat.astype(np.int16).reshape(-1, 16).T
    return np.ascontiguousarray(np.tile(wrapped, (8, 1)))


def prepare(inputs):
    """Build (nc, in_maps) for the 8 cores from full unsharded inputs."""
    bf = ml_dtypes.bfloat16
    sentence = np.asarray(inputs["sentence"])
    emb = np.asarray(inputs["emb"], np.float32)
    Wx_fw = np.asarray(inputs["Wx_fw"], np.float32)
    Wh_fw = np.asarray(inputs["Wh_fw"], np.float32)
    b_fw = np.asarray(inputs["b_fw"], np.float32)
    Wx_bw = np.asarray(inputs["Wx_bw"], np.float32)
    Wh_bw = np.asarray(inputs["Wh_bw"], np.float32)
    b_bw = np.asarray(inputs["b_bw"], np.float32)
    W1 = np.asarray(inputs["W1"], np.float32)
    b1 = np.asarray(inputs["b1"], np.float32)
    W2 = np.asarray(inputs["W2"], np.float32)
    b2 = np.asarray(inputs["b2"], np.float32)

    with_gate_bias = bool(np.any(b_fw) or np.any(b_bw))
    with_b2 = bool(np.any(b2))
    nc = _get_program(with_gate_bias, with_b2)

    emb16 = np.ascontiguousarray(emb.astype(bf))
    wx_f, wh_f, bg_f = _pack_w(Wx_fw, Wh_fw, b_fw)
    wx_b, wh_b, bg_b = _pack_w(Wx_bw, Wh_bw, b_bw)
    w1f = np.ascontiguousarray(W1[0:U].reshape(NK_H, 128, 64).astype(bf))
    w1b = np.ascontiguousarray(W1[U:2 * U].reshape(NK_H, 128, 64).astype(bf))
    w2p = np.ascontiguousarray(W2.astype(bf))
    b1bc = np.ascontiguousarray(np.broadcast_to(b1[None, :], (128, 64)).astype(np.float32))

    in_maps = []
    for c in range(NCORES):
        fw = c < 4
        rows = slice(128 * (c % 4), 128 * (c % 4) + 128)
        toks = sentence[rows][:, :T]
        if not fw:
            toks = toks[:, ::-1]
        flat = np.ascontiguousarray(toks.T).reshape(-1)  # t-major
        m = {
            "emb16": emb16,
            "idx16": _make_idx(flat),
            "wx": wx_f if fw else wx_b,
            "wh": wh_f if fw else wh_b,
            "w1h": w1f if fw else w1b,
            "w2": w2p,
            "b1bc": b1bc,
        }
        if with_gate_bias:
            bg = bg_f if fw else bg_b
            m["bgbc"] = np.ascontiguousarray(
                np.broadcast_to(bg[None, :], (128, G4)).astype(np.float32)
            )
        if with_b2:
            m["b2bc"] = np.ascontiguousarray(
                np.broadcast_to(b2[None, :], (128, V)).astype(np.float32)
            )
        in_maps.append(m)
    return nc, in_maps


def kernel(**inputs):
    from concourse.bass_utils import run_bass_kernel_spmd

    nc, in_maps = prepare(inputs)
    res = run_bass_kernel_spmd(
        nc, in_maps, core_ids=list(range(NCORES)),
        trace=bool(int(os.environ.get("KERNEL_TRACE", "0"))),
    )
    out = np.concatenate([res.results[c]["out"] for c in range(4)], axis=0)
    kernel.last_results = res
    return out.astype(np.float32)
